# revision 18
# baseline (speedup 1.0000x reference)
"""CondConv2D Trainium2 kernel (v3).

Problem (hardcoded): B=16, C_in=64, H=W=256, E=4, C_out=64, 3x3, s=1, d=1, p=1.
Sharding: data-parallel over batch, 8 cores x 2 images.

v3 changes vs v2:
  - bf16 HBM I/O. Input is host-cast to bf16 in the padded tile layout
    (row AND col pads baked host-side), so loads are pure 128x6.7KB
    block copies: no device casts, no pad memsets, half the read
    traffic. Output dram tensor is bf16 (host upcasts to fp32): half
    the write traffic. Total HBM traffic 69MB -> 35MB.
  - Pooling is done by dedicated reduce ops split across DVE
    (tensor_reduce) and ACT (activation-Copy with accum_out into a
    scratch tile), interleaved with the loads / previous image's conv.
  - Routing fixed to sum top+bottom halves (4 ones-matmuls with
    cross tile_positions) instead of per-half logits.
  - Image 1 loads eagerly behind image 0 (bf16 halves the bandwidth
    demand, so loads+stores fit under conv0), its pooling runs during
    conv0, and routing(1) is emitted near conv0's tail so conv1 starts
    with only a ~1us PE bubble.
  - PSUM drains alternate ACT (px0) / DVE (px1).
"""
import sys

if "/opt/trn_rl_repo" not in sys.path:
    sys.path.insert(0, "/opt/trn_rl_repo")

import ml_dtypes
import numpy as np

import concourse.bacc as bacc
import concourse.mybir as mybir
import concourse.tile as tile
from concourse.bass_utils import run_bass_kernel_spmd

F32 = mybir.dt.float32
BF16 = mybir.dt.bfloat16
AF = mybir.ActivationFunctionType
ALU = mybir.AluOpType
XY = mybir.AxisListType.XY

import os

Y_BF16 = os.environ.get("KV3_Y_BF16", "1") == "1"
X_BF16 = os.environ.get("KV3_X_BF16", "1") == "1"

N_CORES = 8
IMGS = 2
C_IN = 64
C_OUT = 64
H = 256
W = 256
E = 4
NTAP = 9
S = NTAP * C_OUT   # 576
RPT = 13           # rows per tile
NT = 10            # tiles per image (130 rows per half: -1..128 / 127..256)
HALF = 128
WP = 258           # padded row width
STAGE_ROWS = 16


def build_nc():
    nc = bacc.Bacc("TRN2", target_bir_lowering=False, debug=False,
                   num_devices=N_CORES)
    # xb: host-prepared bf16 tile layout, pads baked. Partition p<64:
    # top-half channels, row r = x row r-1 (row 0 = zero); p>=64:
    # bottom-half channels, row r = x row 127+r (row 129 = zero).
    # Col 0 and col 257 are zero.
    x = nc.dram_tensor("xb", [IMGS, 128, NT * RPT, WP],
                       BF16 if X_BF16 else F32, kind="ExternalInput")
    wt = nc.dram_tensor("wt", [128, E * S], BF16, kind="ExternalInput")
    fcw = nc.dram_tensor("fcw", [128, E], F32, kind="ExternalInput")
    fcb = nc.dram_tensor("fcb", [128, E], F32, kind="ExternalInput")
    # Device-layout output: partition-major, host unscrambles.
    # Row R' = 4g+2b+j of partition p: see _unscramble_y.
    y = nc.dram_tensor("y", [IMGS, 128, H // 2, W],
                       BF16 if Y_BF16 else F32, kind="ExternalOutput")

    with tile.TileContext(nc) as tc:
        with (
            tc.tile_pool(name="consts", bufs=1) as consts,
            tc.tile_pool(name="small", bufs=2) as small,
            tc.tile_pool(name="scratch", bufs=2) as scratch,
            tc.tile_pool(name="stage", bufs=2) as stage_pool,
            tc.tile_pool(name="psum", bufs=1, space="PSUM") as psum_pool,
        ):
            # ---- tiles (declared up front, loads emitted in priority
            # order below so image 0's first tiles hit the DMA queue
            # first) ----
            wtb = consts.tile([128, E * S], BF16)
            fcwt = consts.tile([128, E], F32)
            fcbt = consts.tile([128, E], F32)
            onest = consts.tile([128, 64], F32)
            nc.vector.memset(onest[:], 1.0)

            # prime the ACT table set (Sigmoid+Copy) during the fill
            sgz = consts.tile([128, 1], F32)
            sgo = consts.tile([128, 1], F32)
            nc.vector.memset(sgz[:], 0.0)
            nc.scalar.activation(sgo[:], sgz[:], AF.Sigmoid)

            # ---- persistent image tiles (loaded fully padded) ----
            xs = [[consts.tile([128, RPT, WP], BF16, name=f"xs{i}_{t}")
                   for t in range(NT)] for i in range(IMGS)]

            # per-image pooling partials: 21 cols used
            partials = [small.tile([128, 24], F32, name=f"par{i}", tag="par",
                                   bufs=2) for i in range(IMGS)]
            for i in range(IMGS):
                nc.vector.memset(partials[i][:], 0.0)

            def load_tile(i, t):
                if X_BF16:
                    nc.sync.dma_start(xs[i][t][:],
                                      x[i, :, RPT * t:RPT * (t + 1), :])
                else:
                    # SWDGE cast-on-load (bisection variant)
                    nc.gpsimd.dma_start(xs[i][t][:],
                                        x[i, :, RPT * t:RPT * (t + 1), :])

            def pool_tile(i, t):
                """Sum tile t of image i into partials[i], split half/half
                across DVE and ACT so per-tile latency tracks the load
                rate. Tile 0: bottom rows 0,1 are x rows 127,128, already
                counted by the top half - excluded."""
                par = partials[i]
                if t == 0:
                    nc.vector.reduce_sum(par[:, 0:1], xs[i][0][:, 2:8, :],
                                         axis=XY)
                    dum = scratch.tile([128, 5, WP], BF16, name="dum0",
                                       tag="dum0", bufs=2)
                    nc.scalar.activation(dum[:], xs[i][0][:, 8:13, :],
                                         AF.Copy, accum_out=par[:, 1:2])
                    nc.vector.reduce_sum(par[0:64, 2:3],
                                         xs[i][0][0:64, 0:2, :], axis=XY)
                else:
                    nc.vector.reduce_sum(par[:, 2 * t + 1:2 * t + 2],
                                         xs[i][t][:, 0:6, :], axis=XY)
                    dum = scratch.tile([128, 7, WP], BF16, name="dum1",
                                       tag="dum1", bufs=2)
                    nc.scalar.activation(dum[:], xs[i][t][:, 6:13, :],
                                         AF.Copy,
                                         accum_out=par[:, 2 * t + 2:2 * t + 3])

            def routing_rt(i):
                """partials -> routing sigmoid tile rt [128, E]."""
                par = partials[i]
                pooled = small.tile([128, 1], F32, name="pooled")
                nc.vector.reduce_sum(pooled[:], par[:, 0:21],
                                     axis=mybir.AxisListType.X)
                tmp4 = small.tile([128, E], F32, name="tmp4")
                nc.vector.tensor_scalar(tmp4[:], fcwt[:], pooled[:, 0:1],
                                        1.0 / float(H * W),
                                        op0=ALU.mult, op1=ALU.mult)
                ps4 = psum_pool.tile([128, E], F32, name="ps4", tag="rt",
                                     bufs=1)
                # full sum (top+bottom) broadcast to both psum halves
                nc.tensor.matmul(ps4[0:64], onest[0:64, :], tmp4[0:64],
                                 start=True, stop=False, tile_position=(0, 0),
                                 skip_group_check=True)
                nc.tensor.matmul(ps4[0:64], onest[64:128, :], tmp4[64:128],
                                 start=False, stop=True, tile_position=(64, 0),
                                 skip_group_check=True)
                nc.tensor.matmul(ps4[64:128], onest[0:64, :], tmp4[0:64],
                                 start=True, stop=False, tile_position=(0, 64),
                                 skip_group_check=True)
                nc.tensor.matmul(ps4[64:128], onest[64:128, :], tmp4[64:128],
                                 start=False, stop=True,
                                 tile_position=(64, 64), skip_group_check=True)
                logits = small.tile([128, E], F32, name="logits")
                nc.vector.tensor_tensor(logits[:], ps4[:], fcbt[:], op=ALU.add)
                rt = small.tile([128, E], F32, name="rt", tag="rtt", bufs=2)
                nc.scalar.activation(rt[:], logits[:], AF.Sigmoid)
                return rt

            def new_wmix():
                return small.tile([128, S], BF16, name="wmix", tag="wmix",
                                  bufs=2)

            def wmix_step(rt, wmix, e):
                if e == 0:
                    nc.vector.tensor_scalar_mul(wmix[:], wtb[:, 0:S],
                                                rt[:, 0:1])
                else:
                    nc.vector.scalar_tensor_tensor(
                        wmix[:], wtb[:, e * S:(e + 1) * S], rt[:, e:e + 1],
                        wmix[:], op0=ALU.mult, op1=ALU.add)

            def conv(i, wmix, on_group=None):
                xi = xs[i]
                n_groups = 32           # 2 pairs per group
                gps = STAGE_ROWS // 4   # groups per stage tile (4)
                stage = None
                for g in range(n_groups):
                    if g % gps == 0:
                        stage = stage_pool.tile([128, STAGE_ROWS, W], BF16,
                                                name="stage", tag="st")
                    psA = psum_pool.tile([128, 2, W], F32, name="psA",
                                         tag="ps", bufs=6)
                    psB = psum_pool.tile([128, 2, W], F32, name="psB",
                                         tag="ps", bufs=6)
                    pstiles = (psA, psB)
                    # last tap must be unsplit for both pairs: pick clean kh
                    bad = set()
                    for px in range(2):
                        pair = 2 * g + px
                        for kh in range(3):
                            if (2 * pair + kh) % RPT == RPT - 1:
                                bad.add(kh)
                    clean = [kh for kh in range(3) if kh not in bad][-1]
                    khs = [kh for kh in range(3) if kh != clean] + [clean]
                    taps = [kh * 3 + kw for kh in khs for kw in range(3)]
                    for r, tap in enumerate(taps):
                        kh, kw = divmod(tap, 3)
                        st = r == 0
                        sp = r == len(taps) - 1
                        for px in range(2):
                            pair = 2 * g + px
                            L = 2 * pair + kh
                            t, m = divmod(L, RPT)
                            ps = pstiles[px]
                            for half in range(2):
                                hs = slice(0, 64) if half == 0 else \
                                    slice(64, 128)
                                lhsT = wmix[hs, tap * 64:(tap + 1) * 64]
                                if px == 0:
                                    tp = (0, 0) if half == 0 else (64, 64)
                                    osl = hs
                                else:
                                    tp = (0, 64) if half == 0 else (64, 0)
                                    osl = slice(64, 128) if half == 0 else \
                                        slice(0, 64)
                                if m <= RPT - 2:
                                    rhs = xi[t][hs, m:m + 2, kw:kw + 256]
                                    nc.tensor.matmul(
                                        ps[osl], lhsT, rhs, start=st, stop=sp,
                                        tile_position=tp,
                                        skip_group_check=True)
                                else:
                                    for j in range(2):
                                        tj, mj = divmod(L + j, RPT)
                                        rhs = xi[tj][hs, mj, kw:kw + 256]
                                        nc.tensor.matmul(
                                            ps[osl, j, :], lhsT, rhs,
                                            start=(st and j == 0), stop=sp,
                                            tile_position=tp,
                                            skip_group_check=True)
                    # drain psum -> staging: px0 on ACT, px1 on DVE
                    r0 = (g % gps) * 4
                    nc.scalar.activation(stage[:, r0:r0 + 2, :], psA[:],
                                         AF.Copy)
                    nc.vector.tensor_copy(stage[:, r0 + 2:r0 + 4, :], psB[:])
                    # stage full -> one contiguous store DMA on gpsimd
                    # (128 descriptors x 8KB; host unscrambles the layout).
                    # Final block of the last image: store per-group so the
                    # end-of-kernel dependency is a 0.25MB DMA, not 1MB.
                    last_blk = i == IMGS - 1 and g >= n_groups - gps
                    if last_blk:
                        mrow = (g // gps) * STAGE_ROWS
                        r0g = (g % gps) * 4
                        nc.gpsimd.dma_start(
                            y[i, :, mrow + r0g:mrow + r0g + 4, :],
                            stage[:, r0g:r0g + 4, :])
                    elif (g + 1) % gps == 0:
                        mrow = (g // gps) * STAGE_ROWS
                        nc.gpsimd.dma_start(
                            y[i, :, mrow:mrow + STAGE_ROWS, :], stage[:])
                    if on_group is not None:
                        on_group(g)

            # ---- schedule ----
            # image 0 tiles 0-1 first, then the small consts, then the
            # rest: tile 0 lands ASAP and consts are in SBUF well before
            # routing(0) needs them.
            load_tile(0, 0)
            load_tile(0, 1)
            nc.sync.dma_start(fcwt[:], fcw[:])
            nc.sync.dma_start(fcbt[:], fcb[:])
            nc.sync.dma_start(wtb[:], wt[:])
            pool_tile(0, 0)
            pool_tile(0, 1)
            for t in range(2, NT):
                load_tile(0, t)
                pool_tile(0, t)
            for t in range(NT):
                load_tile(1, t)
            rt0 = routing_rt(0)
            wmix0 = new_wmix()
            for e in range(E):
                wmix_step(rt0, wmix0, e)

            # image 1 pooling + routing interleaved into conv0's emission
            state = {"rt1": None, "wmix1": new_wmix()}

            def on_group(g):
                if 4 <= g <= 22 and g % 2 == 0:
                    pool_tile(1, (g - 4) // 2)
                elif g == 26:
                    state["rt1"] = routing_rt(1)
                elif g in (28, 29, 30, 31):
                    wmix_step(state["rt1"], state["wmix1"], g - 28)

            conv(0, wmix0, on_group=on_group)
            conv(1, state["wmix1"])

    nc.compile()
    return nc


_NC_CACHE = {}


def _get_nc():
    if "nc" not in _NC_CACHE:
        _NC_CACHE["nc"] = build_nc()
    return _NC_CACHE["nc"]


def _prep_x(x2b):
    """[2, 64, 256, 256] -> padded tile layout [2, 128, 130, 258]."""
    xp = np.zeros((IMGS, 128, NT * RPT, WP),
                  dtype=ml_dtypes.bfloat16 if X_BF16 else np.float32)
    xp[:, 0:64, 1:130, 1:257] = x2b[:, :, 0:129, :]
    xp[:, 64:128, 0:129, 1:257] = x2b[:, :, 127:256, :]
    return xp


def _unscramble_y(ydev):
    """[n, 128, 128, 256] device layout -> [n, 64, 256, 256].

    Device row R' = 4g+2b+j (g conv group, b px, j row-in-pair);
    partition p = P*64+c. b=0: P=0 -> y[c, 4g+j], P=1 -> y[c, 128+4g+j].
    b=1 (px1 psum halves swapped): P=1 -> y[c, 4g+2+j], P=0 -> 128+...
    """
    n = ydev.shape[0]
    ydv = ydev.reshape(n, 2, 64, 32, 2, 2, 256)   # [n, P, c, rr, b, j, w]
    out = np.empty((n, 64, 256, 256), dtype=ydev.dtype)
    yv = out.reshape(n, 64, 2, 32, 4, 256)        # [n, c, H2, rr, cls, w]
    yv[:, :, 0, :, 0:2] = ydv[:, 0, :, :, 0, :]
    yv[:, :, 1, :, 0:2] = ydv[:, 1, :, :, 0, :]
    yv[:, :, 0, :, 2:4] = ydv[:, 1, :, :, 1, :]
    yv[:, :, 1, :, 2:4] = ydv[:, 0, :, :, 1, :]
    return out


def _prep_shared(weight, fc_w, fc_b):
    # [E, O, I, KH, KW] -> [I, E, KH, KW, O] -> [64, E*9*64], dup halves
    wt = np.ascontiguousarray(weight.transpose(2, 0, 3, 4, 1)).reshape(
        C_IN, E * NTAP * C_OUT)
    wt = np.concatenate([wt, wt], axis=0).astype(ml_dtypes.bfloat16)
    fcw = np.concatenate([fc_w.T, fc_w.T], axis=0).astype(np.float32)
    fcb = np.tile(fc_b.reshape(1, E), (128, 1)).astype(np.float32)
    return wt, fcw, fcb


def kernel(inputs, weight, fc_w, fc_b, stride=1, dilation=1, padding=1,
           _trace=False, _npx=2):
    assert int(stride) == 1 and int(dilation) == 1 and int(padding) == 1
    inputs = np.asarray(inputs, dtype=np.float32)
    B = inputs.shape[0]
    assert B == N_CORES * IMGS
    xb = inputs.astype(ml_dtypes.bfloat16) if X_BF16 else inputs
    wt, fcw, fcb = _prep_shared(np.asarray(weight), np.asarray(fc_w),
                                np.asarray(fc_b))
    nc = _get_nc()
    in_maps = []
    for c in range(N_CORES):
        in_maps.append({
            "xb": _prep_x(xb[2 * c:2 * c + 2]),
            "wt": wt, "fcw": fcw, "fcb": fcb,
        })
    res = run_bass_kernel_spmd(nc, in_maps, core_ids=list(range(N_CORES)),
                               trace=_trace)
    ydev = np.concatenate(
        [np.asarray(res.results[c]["y"]) for c in range(N_CORES)], axis=0)
    out = _unscramble_y(ydev).astype(np.float32)
    if _trace:
        return out, res
    return out
